# revision 18
# baseline (speedup 1.0000x reference)
"""nn_Attention_36283883716815 — Bass/Tile kernel for 8 TRN2 NeuronCores.

Sharding: 4 batches x 2 query-row-groups (512 rows each). Each core computes
QKV for its batch (K/V for all 1025 positions incl. sink, Q for its 512 rows),
full causal attention for all 16 heads on its rows, and the output projection.
Rows are host-permuted per core so its own query rows occupy positions 0..511
(SPMD-uniform program; causal block structure is handled with data-side masks).
Only collective: a tiny AllReduce for the magnitude-norm scalars.

Math notes:
 - mp_linear weight prep folds to W~ = W / (0.0032 + ||row||); the eps is
   dropped (1e-4 relative) so 1/||row|| comes from one Abs_reciprocal_sqrt.
   winv is applied at projection-eviction time as a per-partition activation
   scale (outputs are feature-major), not to the weights.
 - HD**-0.25 and 1/sqrt(fan_in) on q,k cancel under unit-normalization.
 - softmax needs no max-subtraction (cosine scores are in [-1,1]); masking is
   multiplicative post-exp; denominator accumulated via a ones-column in V.
 - activation-table discipline: phases A/B use only Square/Identity/ARS (one
   set), phase C uses Exp only, tail uses ARS once.
"""

import numpy as np
import ml_dtypes

B, S, C = 4, 1024, 1024
HD = 64
H = 16
ROT = 32
P = 128
RTC = 4              # row tiles per core
RQ = RTC * P         # 512 query rows per core
SF = S + 1           # 1025 positions incl sink
NCORES = 8

_BF16 = ml_dtypes.bfloat16

_STATE = {}


def _rope_tables():
    inv_freq = 1.0 / (10000.0 ** (np.arange(0, ROT, 2, dtype=np.float32) / ROT))
    t = np.arange(S, dtype=np.float32)
    freqs = np.einsum("i,j->ij", t, inv_freq)        # (S, 16)
    c = np.cos(freqs).T.astype(np.float32)           # (16, S)
    s = np.sin(freqs).T.astype(np.float32)           # (16, S)
    ct32 = np.concatenate([c, c], axis=0)            # (32, S)
    st32 = np.concatenate([-s, s], axis=0)           # (32, S)
    return ct32, st32


def _perm_for(half):
    own = [2 * i + half for i in range(RTC)]
    other = [2 * i + (1 - half) for i in range(RTC)]
    idx = np.concatenate(
        [np.arange(b * P, (b + 1) * P) for b in own + other]
    )
    return idx


# --------------------------------------------------------------------------
# device program
# --------------------------------------------------------------------------

def _build():
    import concourse.mybir as mybir
    import concourse.tile as tile
    from concourse import bacc

    f32 = mybir.dt.float32
    bf16 = mybir.dt.bfloat16

    nc = bacc.Bacc(
        "TRN2",
        target_bir_lowering=False,
        debug=False,
        enable_asserts=False,
        num_devices=NCORES,
    )

    def din(name, shape, dt):
        return nc.dram_tensor(name, shape, dt, kind="ExternalInput").ap()

    aps = dict(
        xb=din("xb", [S, C], f32),
        sinkv=din("sinkv", [1, C], f32),
        wqkv=din("wqkv", [3 * C, C], f32),
        wout=din("wout", [C, C], f32),
        qct=din("qct", [P, RQ], bf16),
        qst=din("qst", [P, RQ], bf16),
        kct=din("kct", [P, S], bf16),
        kst=din("kst", [P, S], bf16),
        rotm=din("rotm", [P, P], bf16),
        tri=din("tri", [P, P], bf16),
        bnd=din("bnd", [P, P], bf16),
        identb=din("identb", [P, P], bf16),
        identf=din("identf", [P, P], f32),
        blk=din("blk", [P, 2], bf16),
        blkTb=din("blkTb", [2, P], bf16),
        ones1f=din("ones1f", [1, P], f32),
        onescf=din("onescf", [P, 1], f32),
        onescb=din("onescb", [P, 1], bf16),
        out=nc.dram_tensor("out", [RQ, C], f32, kind="ExternalOutput").ap(),
    )

    with tile.TileContext(nc) as tc:
        _emit(tc, nc, aps)
    nc.compile()
    return nc


def _emit(tc, nc, t):
    import concourse.mybir as mybir

    f32 = mybir.dt.float32
    bf16 = mybir.dt.bfloat16
    MUL = mybir.AluOpType.mult
    ADD = mybir.AluOpType.add
    EXP = mybir.ActivationFunctionType.Exp
    SQ = mybir.ActivationFunctionType.Square
    IDENT = mybir.ActivationFunctionType.Identity
    ARS = mybir.ActivationFunctionType.Abs_reciprocal_sqrt
    X = mybir.AxisListType.X

    # ---------------- pools ----------------
    const = tc.alloc_tile_pool(name="const", bufs=1)
    persist = tc.alloc_tile_pool(name="persist", bufs=1)
    wsecp = tc.alloc_tile_pool(name="wsecp", bufs=16)   # W.T section rotation
    load = tc.alloc_tile_pool(name="load", bufs=3)
    scratch = tc.alloc_tile_pool(name="scratch", bufs=2)
    small = tc.alloc_tile_pool(name="small", bufs=2)
    rp = tc.alloc_tile_pool(name="rp", bufs=2)
    ptp = tc.alloc_tile_pool(name="ptp", bufs=4)
    opool = tc.alloc_tile_pool(name="opool", bufs=2)

    pmm = tc.alloc_tile_pool(name="pmm", bufs=3, space="PSUM")
    phh = tc.alloc_tile_pool(name="phh", bufs=2, space="PSUM")
    ptb = tc.alloc_tile_pool(name="ptb", bufs=2, space="PSUM")
    psm = tc.alloc_tile_pool(name="psm", bufs=1, space="PSUM")

    dram = tc.alloc_tile_pool(name="dram", bufs=1, space="DRAM")

    # ---------------- constants into SBUF ----------------
    def cload(key, shape, dt):
        tl = const.tile(shape, dt, name=key)
        nc.sync.dma_start(out=tl, in_=t[key])
        return tl

    qct = cload("qct", [P, RQ], bf16)
    qst = cload("qst", [P, RQ], bf16)
    kct = cload("kct", [P, S], bf16)
    kst = cload("kst", [P, S], bf16)
    rotm = cload("rotm", [P, P], bf16)
    tri = cload("tri", [P, P], bf16)
    bnd = cload("bnd", [P, P], bf16)
    identb = cload("identb", [P, P], bf16)
    identf = cload("identf", [P, P], f32)
    blk = cload("blk", [P, 2], bf16)
    blkTb = cload("blkTb", [2, P], bf16)
    ones1f = cload("ones1f", [1, P], f32)
    onescf = cload("onescf", [P, 1], f32)
    onescb = cload("onescb", [P, 1], bf16)

    # ---------------- persistent SBUF tensors ----------------
    xsT = [persist.tile([P, SF], bf16, name=f"xsT{i}") for i in range(8)]
    woutT = [persist.tile([P, C], bf16, name=f"woutT{i}") for i in range(8)]
    kT = [persist.tile([P, SF], bf16, name=f"kT{i}") for i in range(8)]
    qT = [persist.tile([P, RQ], bf16, name=f"qT{i}") for i in range(8)]
    vsb = [persist.tile([P, H * (HD + 1)], bf16, name=f"v{i}") for i in range(8)]
    vsnk = persist.tile([1, H * (HD + 1)], bf16, name="vsnk")
    hT = [persist.tile([P, RQ], bf16, name=f"hT{i}") for i in range(8)]
    wsq = persist.tile([P, 32], f32, name="wsq")
    winv = persist.tile([P, 32], f32, name="winv")
    winvv65 = [persist.tile([65, HD], bf16, name=f"winvv65_{i}") for i in range(H)]
    xsq = persist.tile([P, 9], f32, name="xsq")
    nrm = persist.tile([1, 2], f32, name="nrm")
    nrm2 = persist.tile([1, 2], f32, name="nrm2")

    nc.gpsimd.memset(xsq, 0.0)

    # ============ PHASE A: x prep + weight chunk prep =============
    def prep_chunk(src_ap, wi, dest_tiles, dcol):
        """DMA-cast one [128, C] weight row-chunk, accumulate row sumsq, and
        PE-transpose the raw bf16 chunk into dest_tiles[ct][:, dcol:+128]."""
        ch = load.tile([P, C], bf16, name="ch", tag="ch")
        nc.gpsimd.dma_start(out=ch, in_=src_ap)
        scr = scratch.tile([P, C], bf16, name="scr", tag="scr")
        nc.scalar.activation(scr, ch, SQ, accum_out=wsq[:, wi : wi + 1])
        for ct in range(8):
            pt = ptb.tile([P, P], bf16, name="pt", tag="pt")
            nc.tensor.transpose(pt, ch[:, ct * P : (ct + 1) * P], identb)
            dst = dest_tiles[ct][:, dcol : dcol + P]
            if (wi + ct) % 2 == 0:
                nc.scalar.copy(dst, pt)
            else:
                nc.vector.tensor_copy(dst, pt)

    # x chunks: cast, sumsq, transpose into xsT
    for ri in range(8):
        ch = load.tile([P, C], bf16, name="xch", tag="ch")
        nc.gpsimd.dma_start(out=ch, in_=t["xb"][ri * P : (ri + 1) * P, :])
        scr = scratch.tile([P, C], bf16, name="xscr", tag="scr")
        nc.scalar.activation(scr, ch, SQ, accum_out=xsq[:, ri : ri + 1])
        for ct in range(8):
            pt = ptb.tile([P, P], bf16, name="xpt", tag="pt")
            nc.tensor.transpose(pt, ch[:, ct * P : (ct + 1) * P], identb)
            nc.vector.tensor_copy(xsT[ct][:, ri * P : (ri + 1) * P], pt)

    # sink row of xs
    snk = const.tile([1, C], bf16, name="snk")
    nc.gpsimd.dma_start(out=snk, in_=t["sinkv"])
    sscr = scratch.tile([1, C], bf16, name="sscr", tag="scr")
    nc.scalar.activation(sscr, snk, SQ, accum_out=xsq[0:1, 8:9])
    for ct in range(8):
        pt = ptb.tile([P, P], bf16, name="spt", tag="pt")
        nc.tensor.transpose(
            pt[0:P, 0:1], snk[0:1, ct * P : (ct + 1) * P], identb[0:1, 0:1]
        )
        nc.vector.tensor_copy(xsT[ct][:, S : S + 1], pt[0:P, 0:1])

    # desired-norm partial: 0.5 * sum(||xs rows||); sqrt(x) = x * ars(x)
    xrs = small.tile([P, 8], f32, name="xrs", tag="xrs")
    nc.scalar.activation(xrs, xsq[:, 0:8], ARS)
    xn = small.tile([P, 8], f32, name="xn", tag="xn")
    nc.vector.tensor_mul(xn, xsq[:, 0:8], xrs)
    snkrs = small.tile([1, 1], f32, name="snkrs", tag="snkrs")
    nc.scalar.activation(snkrs, xsq[0:1, 8:9], ARS)
    snkn = small.tile([1, 1], f32, name="snkn", tag="snkn")
    nc.vector.tensor_mul(snkn, xsq[0:1, 8:9], snkrs)
    pd9 = psm.tile([P, 512], f32, name="pd9", tag="sm")
    nc.tensor.matmul(pd9[0:1, 0:8], lhsT=onescf, rhs=xn, start=True, stop=True)
    dtot = small.tile([1, 1], f32, name="dtot", tag="dtot")
    nc.vector.reduce_sum(dtot, pd9[0:1, 0:8], axis=X)
    dtot2 = small.tile([1, 1], f32, name="dtot2", tag="dtot2")
    nc.vector.tensor_add(dtot2, dtot, snkn)
    nc.vector.tensor_scalar_mul(nrm[0:1, 0:1], dtot2, 0.5)

    def walloc(sec):
        return [
            wsecp.tile([P, C], bf16, name=f"w{sec}T{i}", tag="wsec")
            for i in range(8)
        ]

    # ---- K section: prep + k.T projection (feature-major) ----
    wkT = walloc("k")
    for wi in range(8):
        prep_chunk(t["wqkv"][C + wi * P : C + (wi + 1) * P, :], 8 + wi, wkT,
                   wi * P)
    # weight norms for k available after its 8 chunks; batched ARS per section
    nc.scalar.activation(winv[:, 8:16], wsq[:, 8:16], ARS)
    KCH = ((0, 512), (512, 512), (1024, 1))
    for ft in range(8):
        pss = [pmm.tile([P, 512], f32, name=f"psk{j}", tag="mm")
               for j in range(3)]
        for kt in range(8):
            for j, (c0, cw) in enumerate(KCH):
                nc.tensor.matmul(
                    pss[j][:, :cw],
                    lhsT=wkT[kt][:, ft * P : (ft + 1) * P],
                    rhs=xsT[kt][:, c0 : c0 + cw],
                    start=(kt == 0), stop=(kt == 7),
                )
        for j, (c0, cw) in enumerate(KCH):
            nc.scalar.activation(kT[ft][:, c0 : c0 + cw], pss[j][:, :cw],
                                 IDENT, scale=winv[:, 8 + ft : 9 + ft])

    # ---- Q section: prep + q.T projection (rows 0..511 = own rows) ----
    wqT = walloc("q")
    for wi in range(8):
        prep_chunk(t["wqkv"][wi * P : (wi + 1) * P, :], wi, wqT, wi * P)
    nc.scalar.activation(winv[:, 0:8], wsq[:, 0:8], ARS)
    for ft in range(8):
        ps = pmm.tile([P, 512], f32, name="psq", tag="mm")
        for kt in range(8):
            nc.tensor.matmul(
                ps,
                lhsT=wqT[kt][:, ft * P : (ft + 1) * P],
                rhs=xsT[kt][:, 0:RQ],
                start=(kt == 0), stop=(kt == 7),
            )
        nc.scalar.activation(qT[ft], ps, IDENT, scale=winv[:, ft : ft + 1])

    # ---- V section: prep + v projection (natural, head-major 65-strided) --
    wvT = walloc("v")
    for wi in range(8):
        prep_chunk(t["wqkv"][2 * C + wi * P : 2 * C + (wi + 1) * P, :], 16 + wi,
                   wvT, wi * P)
    nc.scalar.activation(winv[:, 16:24], wsq[:, 16:24], ARS)
    for vt in range(9):
        sink_row = vt == 8
        pr = 1 if sink_row else P
        csl = slice(S, S + 1) if sink_row else slice(vt * P, (vt + 1) * P)
        psv = [pmm.tile([P, 512], f32, name=f"psv{nf}", tag="mm")
               for nf in range(2)]
        for kt in range(8):
            for nf in range(2):
                nc.tensor.matmul(
                    psv[nf][:pr, :],
                    lhsT=xsT[kt][:, csl],
                    rhs=wvT[kt][:, nf * 512 : (nf + 1) * 512],
                    start=(kt == 0), stop=(kt == 7),
                )
        vdst = vsnk if sink_row else vsb[vt]
        for nf in range(2):
            dst = vdst[0:pr, nf * 8 * 65 : nf * 8 * 65 + 520]
            dst = dst.rearrange("p (h x) -> p h x", x=65)[:, :, 0:HD]
            nc.scalar.copy(dst, psv[nf][:pr, :])
    for vt in range(9):
        vdst = vsnk if vt == 8 else vsb[vt]
        pr = 1 if vt == 8 else P
        ones_sl = vdst[0:pr, :].rearrange("p (h x) -> p h x", x=65)[:, :, HD:]
        nc.vector.memset(ones_sl, 1.0)

    # per-head v-weight norm rows at partition 64 (bf16) for the
    # denominator-broadcast lhsT
    for h in range(H):
        nc.gpsimd.dma_start(
            out=winvv65[h][64:65, :],
            in_=winv[64 * (h % 2) : 64 * (h % 2) + HD, 16 + h // 2 : 17 + h // 2],
        )

    # ---- W_out section: prep into persistent woutT ----
    for wi in range(8):
        prep_chunk(t["wout"][wi * P : (wi + 1) * P, :], 24 + wi, woutT, wi * P)
    nc.scalar.activation(winv[:, 24:32], wsq[:, 24:32], ARS)

    # ---- rope on q.T / k.T (in place, first 32 dims of each head) ----
    # rotate-half via PE permutation matmul (rotm); tables are full-height
    # with identity (cos=1) / zero (sin=0) padding.
    for pair in range(8):
        for (tl, ct_t, st_t, n) in ((qT[pair], qct, qst, RQ),
                                    (kT[pair], kct, kst, S)):
            for c0 in range(0, n, 512):
                cw = min(512, n - c0)
                sl = tl[:, c0 : c0 + cw]
                prot = pmm.tile([P, 512], f32, name="prot", tag="mm")
                nc.tensor.matmul(prot[:, :cw], lhsT=rotm, rhs=sl,
                                 start=True, stop=True)
                u = rp.tile([P, 512], bf16, name="u", tag="u")[:, 0:cw]
                nc.vector.tensor_mul(u, sl, ct_t[:, c0 : c0 + cw])
                wv = rp.tile([P, 512], bf16, name="wv", tag="wv")[:, 0:cw]
                nc.vector.tensor_mul(wv, prot[:, :cw], st_t[:, c0 : c0 + cw])
                nc.vector.tensor_add(sl, u, wv)

    # ---- unit-normalize q and k (explicit, via ARS + PE broadcast) ----
    for pair in range(8):
        sq = scratch.tile([P, C], bf16, name="sq", tag="scr")[:, 0:RQ]
        nc.vector.tensor_mul(sq, qT[pair], qT[pair])
        pn = pmm.tile([P, 512], f32, name="pn", tag="mm")
        nc.tensor.matmul(pn[0:2, :], lhsT=blk, rhs=sq, start=True, stop=True)
        qinv = small.tile([2, RQ], bf16, name="qinv", tag="qinv", bufs=2)
        nc.scalar.activation(qinv, pn[0:2, :], ARS)
        pb2 = pmm.tile([P, 512], f32, name="pb2", tag="mm")
        nc.tensor.matmul(pb2, lhsT=blkTb, rhs=qinv, start=True, stop=True)
        nc.vector.tensor_mul(qT[pair], pb2, qT[pair])

        for (c0, cw) in KCH:
            sqk = scratch.tile([P, C], bf16, name="sqk", tag="scr")[:, 0:cw]
            nc.vector.tensor_mul(
                sqk, kT[pair][:, c0 : c0 + cw], kT[pair][:, c0 : c0 + cw]
            )
            pnk = pmm.tile([P, 512], f32, name="pnk", tag="mm")
            nc.tensor.matmul(pnk[0:2, :cw], lhsT=blk, rhs=sqk,
                             start=True, stop=True)
            kinv = small.tile([2, 512], bf16, name="kinv", tag="kinv", bufs=2)
            nc.scalar.activation(kinv[:, 0:cw], pnk[0:2, :cw], ARS)
            pbk = pmm.tile([P, 512], f32, name="pbk", tag="mm")
            nc.tensor.matmul(pbk[:, 0:cw], lhsT=blkTb, rhs=kinv[:, 0:cw],
                             start=True, stop=True)
            nc.vector.tensor_mul(kT[pair][:, c0 : c0 + cw], pbk[:, 0:cw],
                                 kT[pair][:, c0 : c0 + cw])

    # ================= PHASE C: attention =================
    pcs = psm.tile([P, 512], f32, name="pcs", tag="sm")
    for h in range(H):
        pair, w = h // 2, h % 2
        base = 64 * w
        qh = qT[pair][base : base + 64, :]
        ph = phh.tile([P, 512], f32, name="ph", tag="hh")
        for cb in range(8):
            i0 = cb % 4
            N = (RTC - i0) * P
            ps = pmm.tile([P, 512], f32, name="ps", tag="mm")[:, :N]
            nc.tensor.matmul(
                ps,
                lhsT=kT[pair][base : base + 64, cb * P : (cb + 1) * P],
                rhs=qh[:, i0 * P : RQ],
                start=True, stop=True,
            )
            pt = ptp.tile([P, 512], bf16, name="ptl", tag="ptl")[:, :N]
            nc.scalar.activation(pt, ps, EXP)
            nc.vector.tensor_mul(pt[:, 0:P], pt[:, 0:P], tri if cb < 4 else bnd)
            nc.tensor.matmul(
                ph[0:65, i0 * P : RQ],
                lhsT=vsb[cb][:, 65 * h : 65 * h + 65],
                rhs=pt,
                start=(cb == 0), stop=False,
            )
        # sink column
        pss = pmm.tile([P, 512], f32, name="pss", tag="mm")
        nc.tensor.matmul(
            pss[0:1, :], lhsT=kT[pair][base : base + 64, S : S + 1], rhs=qh,
            start=True, stop=True,
        )
        pts = ptp.tile([1, 512], bf16, name="pts", tag="pts")
        nc.scalar.activation(pts, pss[0:1, :], EXP)
        nc.tensor.matmul(
            ph[0:65, :], lhsT=vsnk[0:1, 65 * h : 65 * h + 65], rhs=pts,
            start=False, stop=True,
        )
        # per-head 1/denominator (partition 64); broadcast over the 64
        # feature partitions via PE with the per-head v-weight-norm row as
        # lhsT (folds winv_v for free); evict h-hat to a base-0 temp, then
        # SBUF->SBUF DMA into the pair-packed hT slot (crosses partitions).
        den = small.tile([65, RQ], f32, name="den", tag="den")
        nc.vector.reciprocal(den[64:65, :], ph[64:65, :])
        denb = small.tile([65, RQ], bf16, name="denb", tag="denb")
        nc.vector.tensor_copy(denb[64:65, :], den[64:65, :])
        pbc = pmm.tile([P, 512], f32, name="pbc", tag="mm")
        nc.tensor.matmul(
            pbc[0:64, :], lhsT=winvv65[h][64:65, :], rhs=denb[64:65, :],
            start=True, stop=True,
        )
        bcs = ptp.tile([64, 512], bf16, name="bcs", tag="bcs")
        nc.scalar.copy(bcs, pbc[0:64, :])
        tmph = ptp.tile([64, RQ], bf16, name="tmph", tag="tmph")
        nc.vector.tensor_mul(tmph, ph[0:64, :], bcs)
        nc.sync.dma_start(out=hT[pair][base : base + 64, :], in_=tmph)
        hsq = scratch.tile([64, 512], bf16, name="hsq", tag="hsq")
        nc.vector.tensor_mul(hsq, tmph, tmph)
        nc.tensor.matmul(
            pcs[0:1, :], lhsT=onescb[0:64, :], rhs=hsq,
            start=(h == 0), stop=(h == H - 1),
        )

    # current-norm partial: sum_r sqrt(cs_r) with sqrt(x) = x * ars(x)
    crs = small.tile([1, 512], f32, name="crs", tag="crs", bufs=1)
    nc.scalar.activation(crs, pcs[0:1, :], ARS)
    csc = small.tile([1, 512], f32, name="csc", tag="csc", bufs=1)
    nc.vector.scalar_tensor_tensor(
        out=csc, in0=pcs[0:1, :], scalar=1.0, in1=crs, op0=MUL, op1=MUL,
        accum_out=nrm[0:1, 1:2],
    )

    # ================= AllReduce of (desired, current) =================
    ccin = dram.tile([1, 16], f32, name="ccin")
    ccout = dram.tile([1, 16], f32, name="ccout")
    nrmp = persist.tile([1, 16], f32, name="nrmp")
    nc.gpsimd.memset(nrmp, 0.0)
    nc.vector.tensor_copy(nrmp[0:1, 0:2], nrm)
    nc.gpsimd.dma_start(out=ccin, in_=nrmp)
    nc.gpsimd.collective_compute(
        "AllReduce",
        mybir.AluOpType.add,
        replica_groups=[list(range(NCORES))],
        ins=[ccin.opt()],
        outs=[ccout.opt()],
    )
    nrm2p = persist.tile([1, 16], f32, name="nrm2p")
    nc.gpsimd.dma_start(out=nrm2p, in_=ccout)
    nc.vector.tensor_copy(nrm2, nrm2p[0:1, 0:2])

    magr = small.tile([1, 1], f32, name="magr", tag="magr")
    nc.vector.reciprocal(magr, nrm2[0:1, 1:2])
    mag = small.tile([1, 1], f32, name="mag", tag="mag")
    nc.vector.scalar_tensor_tensor(
        out=mag, in0=nrm2[0:1, 0:1], scalar=float(4096.0 / 4100.0), in1=magr,
        op0=MUL, op1=MUL,
    )
    pmg = psm.tile([P, 512], f32, name="pmg", tag="sm")
    nc.tensor.matmul(pmg[0:P, 0:1], lhsT=ones1f, rhs=mag, start=True, stop=True)
    magb = small.tile([P, 1], f32, name="magb", tag="magb")
    nc.vector.tensor_copy(magb, pmg[0:P, 0:1])

    # ================= PHASE D: output projection =================
    for ot in range(8):
        pp = pmm.tile([P, 512], f32, name="pp", tag="mm")
        for ift in range(8):
            nc.tensor.matmul(
                pp,
                lhsT=woutT[ift][:, ot * P : (ot + 1) * P],
                rhs=hT[ift],
                start=(ift == 0), stop=(ift == 7),
            )
        po = opool.tile([P, 512], f32, name="po", tag="po")
        nc.scalar.activation(po, pp, IDENT, scale=winv[:, 24 + ot : 25 + ot])
        for i in range(RTC):
            pt2 = ptb.tile([P, P], f32, name="pt2", tag="pt")
            nc.tensor.matmul(
                pt2, lhsT=po[:, i * P : (i + 1) * P], rhs=identf,
                is_transpose=True,
            )
            ost = opool.tile([P, P], f32, name="ost", tag="ost")
            nc.vector.tensor_scalar_mul(ost, pt2, magb)
            nc.sync.dma_start(
                out=t["out"][i * P : (i + 1) * P, ot * P : (ot + 1) * P],
                in_=ost,
            )

    for p in reversed((const, persist, wsecp, load, scratch, small, rp, ptp,
                       opool, pmm, phh, ptb, psm, dram)):
        p.release()


# --------------------------------------------------------------------------
# host wrapper
# --------------------------------------------------------------------------

def _get_program():
    if "nc" not in _STATE:
        _STATE["nc"] = _build()
    return _STATE["nc"]


def _full_tables(ct32, st32, idx):
    n = len(idx)
    ct = np.ones((P, n), np.float32)
    st = np.zeros((P, n), np.float32)
    for blk in (0, 64):
        ct[blk : blk + 32] = ct32[:, idx]
        st[blk : blk + 32] = st32[:, idx]
    return ct.astype(_BF16), st.astype(_BF16)


def _make_in_maps(x, re, attn_mask, w_qkv, w_out, sink):
    ct32, st32 = _rope_tables()
    rotm = np.zeros((P, P), np.float32)
    for blk in (0, 64):
        for d in range(16):
            rotm[blk + d, blk + d + 16] = 1.0
            rotm[blk + d + 16, blk + d] = 1.0
    tri = np.triu(np.ones((P, P), np.float32))  # [j, r]: keep j <= r
    identf = np.eye(P, dtype=np.float32)
    blk = np.zeros((P, 2), np.float32)
    blk[0:64, 0] = 1.0
    blk[64:128, 1] = 1.0
    consts = {
        "tri": tri.astype(_BF16),
        "identb": identf.astype(_BF16),
        "identf": identf,
        "blk": blk.astype(_BF16),
        "blkTb": np.ascontiguousarray(blk.T).astype(_BF16),
        "ones1f": np.ones((1, P), np.float32),
        "onescf": np.ones((P, 1), np.float32),
        "onescb": np.ones((P, 1), _BF16),
        "rotm": rotm.astype(_BF16),
        "wqkv": np.ascontiguousarray(w_qkv, dtype=np.float32),
        "wout": np.ascontiguousarray(w_out, dtype=np.float32),
        "sinkv": np.ascontiguousarray(sink.reshape(1, C), dtype=np.float32),
    }
    in_maps = []
    perms = []
    for c in range(NCORES):
        b, half = c // 2, c % 2
        idx = _perm_for(half)
        perms.append((b, idx))
        m = dict(consts)
        m["xb"] = np.ascontiguousarray(x[b][idx], dtype=np.float32)
        m["qct"], m["qst"] = _full_tables(ct32, st32, idx[:RQ])
        m["kct"], m["kst"] = _full_tables(ct32, st32, idx)
        m["bnd"] = np.full((P, P), float(half), np.float32).astype(_BF16)
        in_maps.append(m)
    return in_maps, perms


def _run(inputs, trace=False):
    from concourse.bass_utils import run_bass_kernel_spmd

    nc = _get_program()
    in_maps, perms = _make_in_maps(**inputs)
    res = run_bass_kernel_spmd(
        nc, in_maps, core_ids=list(range(NCORES)), trace=trace
    )
    full = np.empty((B, S, C), np.float32)
    for c in range(NCORES):
        b, idx = perms[c]
        full[b][idx[:RQ]] = res.results[c]["out"]
    return full, res


def kernel(x, re, attn_mask, w_qkv, w_out, sink):
    inputs = dict(x=np.asarray(x), re=np.asarray(re),
                  attn_mask=np.asarray(attn_mask),
                  w_qkv=np.asarray(w_qkv), w_out=np.asarray(w_out),
                  sink=np.asarray(sink))
    full, _ = _run(inputs)
    return full


# revision 19
# speedup vs baseline: 1.0173x; 1.0173x over previous
"""nn_Attention_36283883716815 — Bass/Tile kernel for 8 TRN2 NeuronCores.

Sharding: 4 batches x 2 query-row-groups (512 rows each). Each core computes
QKV for its batch (K/V for all 1025 positions incl. sink, Q for its 512 rows),
full causal attention for all 16 heads on its rows, and the output projection.
Rows are host-permuted per core so its own query rows occupy positions 0..511
(SPMD-uniform program; causal block structure is handled with data-side masks).
Only collective: a tiny AllReduce for the magnitude-norm scalars.

Math notes:
 - mp_linear weight prep folds to W~ = W / (0.0032 + ||row||); the eps is
   dropped (1e-4 relative) so 1/||row|| comes from one Abs_reciprocal_sqrt.
   winv is applied at projection-eviction time as a per-partition activation
   scale (outputs are feature-major), not to the weights.
 - HD**-0.25 and 1/sqrt(fan_in) on q,k cancel under unit-normalization.
 - softmax needs no max-subtraction (cosine scores are in [-1,1]); masking is
   multiplicative post-exp; denominator accumulated via a ones-column in V.
 - activation-table discipline: phases A/B use only Square/Identity/ARS (one
   set), phase C uses Exp only, tail uses ARS once.
"""

import numpy as np
import ml_dtypes

B, S, C = 4, 1024, 1024
HD = 64
H = 16
ROT = 32
P = 128
RTC = 4              # row tiles per core
RQ = RTC * P         # 512 query rows per core
SF = S + 1           # 1025 positions incl sink
NCORES = 8

_BF16 = ml_dtypes.bfloat16

_STATE = {}


def _rope_tables():
    inv_freq = 1.0 / (10000.0 ** (np.arange(0, ROT, 2, dtype=np.float32) / ROT))
    t = np.arange(S, dtype=np.float32)
    freqs = np.einsum("i,j->ij", t, inv_freq)        # (S, 16)
    c = np.cos(freqs).T.astype(np.float32)           # (16, S)
    s = np.sin(freqs).T.astype(np.float32)           # (16, S)
    ct32 = np.concatenate([c, c], axis=0)            # (32, S)
    st32 = np.concatenate([-s, s], axis=0)           # (32, S)
    return ct32, st32


def _perm_for(half):
    own = [2 * i + half for i in range(RTC)]
    other = [2 * i + (1 - half) for i in range(RTC)]
    idx = np.concatenate(
        [np.arange(b * P, (b + 1) * P) for b in own + other]
    )
    return idx


# --------------------------------------------------------------------------
# device program
# --------------------------------------------------------------------------

def _build():
    import concourse.mybir as mybir
    import concourse.tile as tile
    from concourse import bacc

    f32 = mybir.dt.float32
    bf16 = mybir.dt.bfloat16

    nc = bacc.Bacc(
        "TRN2",
        target_bir_lowering=False,
        debug=False,
        enable_asserts=False,
        num_devices=NCORES,
    )

    def din(name, shape, dt):
        return nc.dram_tensor(name, shape, dt, kind="ExternalInput").ap()

    aps = dict(
        xb=din("xb", [S, C], f32),
        sinkv=din("sinkv", [1, C], f32),
        wqkv=din("wqkv", [3 * C, C], f32),
        wout=din("wout", [C, C], f32),
        qct=din("qct", [P, RQ], bf16),
        qst=din("qst", [P, RQ], bf16),
        kct=din("kct", [P, S], bf16),
        kst=din("kst", [P, S], bf16),
        rotm=din("rotm", [P, P], bf16),
        tri=din("tri", [P, P], bf16),
        bnd=din("bnd", [P, P], bf16),
        identb=din("identb", [P, P], bf16),
        identf=din("identf", [P, P], f32),
        blk=din("blk", [P, 2], bf16),
        blkTb=din("blkTb", [2, P], bf16),
        ones1f=din("ones1f", [1, P], f32),
        onescf=din("onescf", [P, 1], f32),
        onescb=din("onescb", [P, 1], bf16),
        out=nc.dram_tensor("out", [RQ, C], f32, kind="ExternalOutput").ap(),
    )

    with tile.TileContext(nc) as tc:
        _emit(tc, nc, aps)
    nc.compile()
    return nc


def _emit(tc, nc, t):
    import concourse.mybir as mybir

    f32 = mybir.dt.float32
    bf16 = mybir.dt.bfloat16
    MUL = mybir.AluOpType.mult
    ADD = mybir.AluOpType.add
    EXP = mybir.ActivationFunctionType.Exp
    SQ = mybir.ActivationFunctionType.Square
    IDENT = mybir.ActivationFunctionType.Identity
    ARS = mybir.ActivationFunctionType.Abs_reciprocal_sqrt
    X = mybir.AxisListType.X

    # ---------------- pools ----------------
    const = tc.alloc_tile_pool(name="const", bufs=1)
    persist = tc.alloc_tile_pool(name="persist", bufs=1)
    wsecp = tc.alloc_tile_pool(name="wsecp", bufs=16)   # W.T section rotation
    load = tc.alloc_tile_pool(name="load", bufs=3)
    scratch = tc.alloc_tile_pool(name="scratch", bufs=2)
    small = tc.alloc_tile_pool(name="small", bufs=2)
    rp = tc.alloc_tile_pool(name="rp", bufs=2)
    ptp = tc.alloc_tile_pool(name="ptp", bufs=6)
    opool = tc.alloc_tile_pool(name="opool", bufs=2)

    pmm = tc.alloc_tile_pool(name="pmm", bufs=3, space="PSUM")
    phh = tc.alloc_tile_pool(name="phh", bufs=2, space="PSUM")
    ptb = tc.alloc_tile_pool(name="ptb", bufs=2, space="PSUM")
    psm = tc.alloc_tile_pool(name="psm", bufs=1, space="PSUM")

    dram = tc.alloc_tile_pool(name="dram", bufs=1, space="DRAM")

    # ---------------- constants into SBUF ----------------
    def cload(key, shape, dt):
        tl = const.tile(shape, dt, name=key)
        nc.sync.dma_start(out=tl, in_=t[key])
        return tl

    qct = cload("qct", [P, RQ], bf16)
    qst = cload("qst", [P, RQ], bf16)
    kct = cload("kct", [P, S], bf16)
    kst = cload("kst", [P, S], bf16)
    rotm = cload("rotm", [P, P], bf16)
    tri = cload("tri", [P, P], bf16)
    bnd = cload("bnd", [P, P], bf16)
    identb = cload("identb", [P, P], bf16)
    identf = cload("identf", [P, P], f32)
    blk = cload("blk", [P, 2], bf16)
    blkTb = cload("blkTb", [2, P], bf16)
    ones1f = cload("ones1f", [1, P], f32)
    onescf = cload("onescf", [P, 1], f32)
    onescb = cload("onescb", [P, 1], bf16)

    # ---------------- persistent SBUF tensors ----------------
    xsT = [persist.tile([P, SF], bf16, name=f"xsT{i}") for i in range(8)]
    woutT = [persist.tile([P, C], bf16, name=f"woutT{i}") for i in range(8)]
    kT = [persist.tile([P, SF], bf16, name=f"kT{i}") for i in range(8)]
    qT = [persist.tile([P, RQ], bf16, name=f"qT{i}") for i in range(8)]
    vsb = [persist.tile([P, H * (HD + 1)], bf16, name=f"v{i}") for i in range(8)]
    vsnk = persist.tile([1, H * (HD + 1)], bf16, name="vsnk")
    hT = [persist.tile([P, RQ], bf16, name=f"hT{i}") for i in range(8)]
    wsq = persist.tile([P, 32], f32, name="wsq")
    winv = persist.tile([P, 32], f32, name="winv")
    winvv65 = [persist.tile([65, HD], bf16, name=f"winvv65_{i}") for i in range(H)]
    xsq = persist.tile([P, 9], f32, name="xsq")
    nrm = persist.tile([1, 2], f32, name="nrm")
    nrm2 = persist.tile([1, 2], f32, name="nrm2")
    orow = [persist.tile([P, C], f32, name=f"orow{i}") for i in range(RTC)]

    nc.gpsimd.memset(xsq, 0.0)

    # ============ PHASE A: x prep + weight chunk prep =============
    def prep_chunk(src_ap, wi, dest_tiles, dcol):
        """DMA-cast one [128, C] weight row-chunk, accumulate row sumsq, and
        PE-transpose the raw bf16 chunk into dest_tiles[ct][:, dcol:+128]."""
        ch = load.tile([P, C], bf16, name="ch", tag="ch")
        nc.gpsimd.dma_start(out=ch, in_=src_ap)
        scr = scratch.tile([P, C], bf16, name="scr", tag="scr")
        nc.scalar.activation(scr, ch, SQ, accum_out=wsq[:, wi : wi + 1])
        for ct in range(8):
            pt = ptb.tile([P, P], bf16, name="pt", tag="pt")
            nc.tensor.transpose(pt, ch[:, ct * P : (ct + 1) * P], identb)
            dst = dest_tiles[ct][:, dcol : dcol + P]
            if (wi + ct) % 2 == 0:
                nc.scalar.copy(dst, pt)
            else:
                nc.vector.tensor_copy(dst, pt)

    # x chunks: cast, sumsq, transpose into xsT
    for ri in range(8):
        ch = load.tile([P, C], bf16, name="xch", tag="ch")
        nc.gpsimd.dma_start(out=ch, in_=t["xb"][ri * P : (ri + 1) * P, :])
        scr = scratch.tile([P, C], bf16, name="xscr", tag="scr")
        nc.scalar.activation(scr, ch, SQ, accum_out=xsq[:, ri : ri + 1])
        for ct in range(8):
            pt = ptb.tile([P, P], bf16, name="xpt", tag="pt")
            nc.tensor.transpose(pt, ch[:, ct * P : (ct + 1) * P], identb)
            nc.vector.tensor_copy(xsT[ct][:, ri * P : (ri + 1) * P], pt)

    # sink row of xs
    snk = const.tile([1, C], bf16, name="snk")
    nc.gpsimd.dma_start(out=snk, in_=t["sinkv"])
    sscr = scratch.tile([1, C], bf16, name="sscr", tag="scr")
    nc.scalar.activation(sscr, snk, SQ, accum_out=xsq[0:1, 8:9])
    for ct in range(8):
        pt = ptb.tile([P, P], bf16, name="spt", tag="pt")
        nc.tensor.transpose(
            pt[0:P, 0:1], snk[0:1, ct * P : (ct + 1) * P], identb[0:1, 0:1]
        )
        nc.vector.tensor_copy(xsT[ct][:, S : S + 1], pt[0:P, 0:1])

    # desired-norm partial: 0.5 * sum(||xs rows||); sqrt(x) = x * ars(x)
    xrs = small.tile([P, 8], f32, name="xrs", tag="xrs")
    nc.scalar.activation(xrs, xsq[:, 0:8], ARS)
    xn = small.tile([P, 8], f32, name="xn", tag="xn")
    nc.vector.tensor_mul(xn, xsq[:, 0:8], xrs)
    snkrs = small.tile([1, 1], f32, name="snkrs", tag="snkrs")
    nc.scalar.activation(snkrs, xsq[0:1, 8:9], ARS)
    snkn = small.tile([1, 1], f32, name="snkn", tag="snkn")
    nc.vector.tensor_mul(snkn, xsq[0:1, 8:9], snkrs)
    pd9 = psm.tile([P, 512], f32, name="pd9", tag="sm")
    nc.tensor.matmul(pd9[0:1, 0:8], lhsT=onescf, rhs=xn, start=True, stop=True)
    dtot = small.tile([1, 1], f32, name="dtot", tag="dtot")
    nc.vector.reduce_sum(dtot, pd9[0:1, 0:8], axis=X)
    dtot2 = small.tile([1, 1], f32, name="dtot2", tag="dtot2")
    nc.vector.tensor_add(dtot2, dtot, snkn)
    nc.vector.tensor_scalar_mul(nrm[0:1, 0:1], dtot2, 0.5)

    def walloc(sec):
        return [
            wsecp.tile([P, C], bf16, name=f"w{sec}T{i}", tag="wsec")
            for i in range(8)
        ]

    # ---- K section: prep + k.T projection (feature-major) ----
    wkT = walloc("k")
    for wi in range(8):
        prep_chunk(t["wqkv"][C + wi * P : C + (wi + 1) * P, :], 8 + wi, wkT,
                   wi * P)
    # weight norms for k available after its 8 chunks; batched ARS per section
    nc.scalar.activation(winv[:, 8:16], wsq[:, 8:16], ARS)
    KCH = ((0, 512), (512, 512), (1024, 1))
    for ft in range(8):
        pss = [pmm.tile([P, 512], f32, name=f"psk{j}", tag="mm")
               for j in range(3)]
        for kt in range(8):
            for j, (c0, cw) in enumerate(KCH):
                nc.tensor.matmul(
                    pss[j][:, :cw],
                    lhsT=wkT[kt][:, ft * P : (ft + 1) * P],
                    rhs=xsT[kt][:, c0 : c0 + cw],
                    start=(kt == 0), stop=(kt == 7),
                )
        for j, (c0, cw) in enumerate(KCH):
            nc.scalar.activation(kT[ft][:, c0 : c0 + cw], pss[j][:, :cw],
                                 IDENT, scale=winv[:, 8 + ft : 9 + ft])

    # ---- Q section: prep + q.T projection (rows 0..511 = own rows) ----
    wqT = walloc("q")
    for wi in range(8):
        prep_chunk(t["wqkv"][wi * P : (wi + 1) * P, :], wi, wqT, wi * P)
    nc.scalar.activation(winv[:, 0:8], wsq[:, 0:8], ARS)
    for ft in range(8):
        ps = pmm.tile([P, 512], f32, name="psq", tag="mm")
        for kt in range(8):
            nc.tensor.matmul(
                ps,
                lhsT=wqT[kt][:, ft * P : (ft + 1) * P],
                rhs=xsT[kt][:, 0:RQ],
                start=(kt == 0), stop=(kt == 7),
            )
        nc.scalar.activation(qT[ft], ps, IDENT, scale=winv[:, ft : ft + 1])

    # ---- V section: prep + v projection (natural, head-major 65-strided) --
    wvT = walloc("v")
    for wi in range(8):
        prep_chunk(t["wqkv"][2 * C + wi * P : 2 * C + (wi + 1) * P, :], 16 + wi,
                   wvT, wi * P)
    nc.scalar.activation(winv[:, 16:24], wsq[:, 16:24], ARS)
    for vt in range(9):
        sink_row = vt == 8
        pr = 1 if sink_row else P
        csl = slice(S, S + 1) if sink_row else slice(vt * P, (vt + 1) * P)
        psv = [pmm.tile([P, 512], f32, name=f"psv{nf}", tag="mm")
               for nf in range(2)]
        for kt in range(8):
            for nf in range(2):
                nc.tensor.matmul(
                    psv[nf][:pr, :],
                    lhsT=xsT[kt][:, csl],
                    rhs=wvT[kt][:, nf * 512 : (nf + 1) * 512],
                    start=(kt == 0), stop=(kt == 7),
                )
        vdst = vsnk if sink_row else vsb[vt]
        for nf in range(2):
            dst = vdst[0:pr, nf * 8 * 65 : nf * 8 * 65 + 520]
            dst = dst.rearrange("p (h x) -> p h x", x=65)[:, :, 0:HD]
            nc.scalar.copy(dst, psv[nf][:pr, :])
    for vt in range(9):
        vdst = vsnk if vt == 8 else vsb[vt]
        pr = 1 if vt == 8 else P
        ones_sl = vdst[0:pr, :].rearrange("p (h x) -> p h x", x=65)[:, :, HD:]
        nc.vector.memset(ones_sl, 1.0)

    # per-head v-weight norm rows at partition 64 (bf16) for the
    # denominator-broadcast lhsT
    for h in range(H):
        nc.gpsimd.dma_start(
            out=winvv65[h][64:65, :],
            in_=winv[64 * (h % 2) : 64 * (h % 2) + HD, 16 + h // 2 : 17 + h // 2],
        )

    # ---- W_out section: prep into persistent woutT ----
    for wi in range(8):
        prep_chunk(t["wout"][wi * P : (wi + 1) * P, :], 24 + wi, woutT, wi * P)
    nc.scalar.activation(winv[:, 24:32], wsq[:, 24:32], ARS)

    # ---- per pair: rope + unit-normalize q,k, then its two heads of
    # attention (keeps PE dense; avoids a HAM re-throttle window) ----
    pcs = psm.tile([P, 512], f32, name="pcs", tag="sm")
    for pair in range(8):
        # rope (rotate-half via PE permutation matmul rotm)
        for (tl, ct_t, st_t, n) in ((qT[pair], qct, qst, RQ),
                                    (kT[pair], kct, kst, S)):
            for c0 in range(0, n, 512):
                cw = min(512, n - c0)
                sl = tl[:, c0 : c0 + cw]
                prot = pmm.tile([P, 512], f32, name="prot", tag="mm")
                nc.tensor.matmul(prot[:, :cw], lhsT=rotm, rhs=sl,
                                 start=True, stop=True)
                u = rp.tile([P, 512], bf16, name="u", tag="u")[:, 0:cw]
                nc.vector.tensor_mul(u, sl, ct_t[:, c0 : c0 + cw])
                wv = rp.tile([P, 512], bf16, name="wv", tag="wv")[:, 0:cw]
                nc.vector.tensor_mul(wv, prot[:, :cw], st_t[:, c0 : c0 + cw])
                nc.vector.tensor_add(sl, u, wv)

        # unit-normalize q and k (ARS + PE broadcast)
        sq = scratch.tile([P, C], bf16, name="sq", tag="scr")[:, 0:RQ]
        nc.vector.tensor_mul(sq, qT[pair], qT[pair])
        pn = pmm.tile([P, 512], f32, name="pn", tag="mm")
        nc.tensor.matmul(pn[0:2, :], lhsT=blk, rhs=sq, start=True, stop=True)
        qinv = small.tile([2, RQ], bf16, name="qinv", tag="qinv", bufs=2)
        nc.scalar.activation(qinv, pn[0:2, :], ARS)
        pb2 = pmm.tile([P, 512], f32, name="pb2", tag="mm")
        nc.tensor.matmul(pb2, lhsT=blkTb, rhs=qinv, start=True, stop=True)
        nc.vector.tensor_mul(qT[pair], pb2, qT[pair])

        for (c0, cw) in KCH:
            sqk = scratch.tile([P, C], bf16, name="sqk", tag="scr")[:, 0:cw]
            nc.vector.tensor_mul(
                sqk, kT[pair][:, c0 : c0 + cw], kT[pair][:, c0 : c0 + cw]
            )
            pnk = pmm.tile([P, 512], f32, name="pnk", tag="mm")
            nc.tensor.matmul(pnk[0:2, :cw], lhsT=blk, rhs=sqk,
                             start=True, stop=True)
            kinv = small.tile([2, 512], bf16, name="kinv", tag="kinv", bufs=2)
            nc.scalar.activation(kinv[:, 0:cw], pnk[0:2, :cw], ARS)
            pbk = pmm.tile([P, 512], f32, name="pbk", tag="mm")
            nc.tensor.matmul(pbk[:, 0:cw], lhsT=blkTb, rhs=kinv[:, 0:cw],
                             start=True, stop=True)
            nc.vector.tensor_mul(kT[pair][:, c0 : c0 + cw], pbk[:, 0:cw],
                                 kT[pair][:, c0 : c0 + cw])

        # ---- attention for this pair's two heads ----
        for w in range(2):
            h = 2 * pair + w
            base = 64 * w
            qh = qT[pair][base : base + 64, :]
            ph = phh.tile([P, 512], f32, name="ph", tag="hh")
            for cb in range(8):
                i0 = cb % 4
                N = (RTC - i0) * P
                ps = pmm.tile([P, 512], f32, name="ps", tag="mm")[:, :N]
                nc.tensor.matmul(
                    ps,
                    lhsT=kT[pair][base : base + 64, cb * P : (cb + 1) * P],
                    rhs=qh[:, i0 * P : RQ],
                    start=True, stop=True,
                )
                pt = ptp.tile([P, 512], bf16, name="ptl", tag="ptl")[:, :N]
                nc.scalar.activation(pt, ps, EXP)
                nc.vector.tensor_mul(pt[:, 0:P], pt[:, 0:P],
                                     tri if cb < 4 else bnd)
                nc.tensor.matmul(
                    ph[0:65, i0 * P : RQ],
                    lhsT=vsb[cb][:, 65 * h : 65 * h + 65],
                    rhs=pt,
                    start=(cb == 0), stop=False,
                )
            # sink column
            pss = pmm.tile([P, 512], f32, name="pss", tag="mm")
            nc.tensor.matmul(
                pss[0:1, :], lhsT=kT[pair][base : base + 64, S : S + 1],
                rhs=qh, start=True, stop=True,
            )
            pts = ptp.tile([1, 512], bf16, name="pts", tag="pts")
            nc.scalar.activation(pts, pss[0:1, :], EXP)
            nc.tensor.matmul(
                ph[0:65, :], lhsT=vsnk[0:1, 65 * h : 65 * h + 65], rhs=pts,
                start=False, stop=True,
            )
            # 1/denominator, winv_v-folded broadcast, h-hat eviction
            den = small.tile([65, RQ], f32, name="den", tag="den")
            nc.vector.reciprocal(den[64:65, :], ph[64:65, :])
            denb = small.tile([65, RQ], bf16, name="denb", tag="denb")
            nc.vector.tensor_copy(denb[64:65, :], den[64:65, :])
            pbc = pmm.tile([P, 512], f32, name="pbc", tag="mm")
            nc.tensor.matmul(
                pbc[0:64, :], lhsT=winvv65[h][64:65, :], rhs=denb[64:65, :],
                start=True, stop=True,
            )
            bcs = ptp.tile([64, 512], bf16, name="bcs", tag="bcs")
            nc.scalar.copy(bcs, pbc[0:64, :])
            tmph = ptp.tile([64, RQ], bf16, name="tmph", tag="tmph")
            nc.vector.tensor_mul(tmph, ph[0:64, :], bcs)
            nc.sync.dma_start(out=hT[pair][base : base + 64, :], in_=tmph)
            hsq = scratch.tile([64, 512], bf16, name="hsq", tag="hsq")
            nc.vector.tensor_mul(hsq, tmph, tmph)
            nc.tensor.matmul(
                pcs[0:1, :], lhsT=onescb[0:64, :], rhs=hsq,
                start=(h == 0), stop=(h == H - 1),
            )

    # current-norm partial: sum_r sqrt(cs_r) with sqrt(x) = x * ars(x)
    crs = small.tile([1, 512], f32, name="crs", tag="crs", bufs=1)
    nc.scalar.activation(crs, pcs[0:1, :], ARS)
    csc = small.tile([1, 512], f32, name="csc", tag="csc", bufs=1)
    nc.vector.scalar_tensor_tensor(
        out=csc, in0=pcs[0:1, :], scalar=1.0, in1=crs, op0=MUL, op1=MUL,
        accum_out=nrm[0:1, 1:2],
    )

    # ================= AllReduce of (desired, current) =================
    ccin = dram.tile([1, 16], f32, name="ccin")
    ccout = dram.tile([1, 16], f32, name="ccout")
    nrmp = persist.tile([1, 16], f32, name="nrmp")
    nc.gpsimd.memset(nrmp, 0.0)
    nc.vector.tensor_copy(nrmp[0:1, 0:2], nrm)
    nc.gpsimd.dma_start(out=ccin, in_=nrmp)
    nc.gpsimd.collective_compute(
        "AllReduce",
        mybir.AluOpType.add,
        replica_groups=[list(range(NCORES))],
        ins=[ccin.opt()],
        outs=[ccout.opt()],
    )
    nrm2p = persist.tile([1, 16], f32, name="nrm2p")
    nc.gpsimd.dma_start(out=nrm2p, in_=ccout)
    nc.vector.tensor_copy(nrm2, nrm2p[0:1, 0:2])

    magr = small.tile([1, 1], f32, name="magr", tag="magr")
    nc.vector.reciprocal(magr, nrm2[0:1, 1:2])
    mag = small.tile([1, 1], f32, name="mag", tag="mag")
    nc.vector.scalar_tensor_tensor(
        out=mag, in0=nrm2[0:1, 0:1], scalar=float(4096.0 / 4100.0), in1=magr,
        op0=MUL, op1=MUL,
    )
    pmg = psm.tile([P, 512], f32, name="pmg", tag="sm")
    nc.tensor.matmul(pmg[0:P, 0:1], lhsT=ones1f, rhs=mag, start=True, stop=True)
    magb = small.tile([P, 1], f32, name="magb", tag="magb")
    nc.vector.tensor_copy(magb, pmg[0:P, 0:1])

    # ================= PHASE D: output projection =================
    for ot in range(8):
        pp = pmm.tile([P, 512], f32, name="pp", tag="mm")
        for ift in range(8):
            nc.tensor.matmul(
                pp,
                lhsT=woutT[ift][:, ot * P : (ot + 1) * P],
                rhs=hT[ift],
                start=(ift == 0), stop=(ift == 7),
            )
        po = opool.tile([P, 512], f32, name="po", tag="po")
        nc.scalar.activation(po, pp, IDENT, scale=winv[:, 24 + ot : 25 + ot])
        for i in range(RTC):
            pt2 = ptb.tile([P, P], f32, name="pt2", tag="pt")
            nc.tensor.matmul(
                pt2, lhsT=po[:, i * P : (i + 1) * P], rhs=identf,
                is_transpose=True,
            )
            nc.vector.tensor_scalar_mul(
                orow[i][:, ot * P : (ot + 1) * P], pt2, magb
            )
    for i in range(RTC):
        nc.sync.dma_start(out=t["out"][i * P : (i + 1) * P, :], in_=orow[i])

    for p in reversed((const, persist, wsecp, load, scratch, small, rp, ptp,
                       opool, pmm, phh, ptb, psm, dram)):
        p.release()


# --------------------------------------------------------------------------
# host wrapper
# --------------------------------------------------------------------------

def _get_program():
    if "nc" not in _STATE:
        _STATE["nc"] = _build()
    return _STATE["nc"]


def _full_tables(ct32, st32, idx):
    n = len(idx)
    ct = np.ones((P, n), np.float32)
    st = np.zeros((P, n), np.float32)
    for blk in (0, 64):
        ct[blk : blk + 32] = ct32[:, idx]
        st[blk : blk + 32] = st32[:, idx]
    return ct.astype(_BF16), st.astype(_BF16)


def _make_in_maps(x, re, attn_mask, w_qkv, w_out, sink):
    ct32, st32 = _rope_tables()
    rotm = np.zeros((P, P), np.float32)
    for blk in (0, 64):
        for d in range(16):
            rotm[blk + d, blk + d + 16] = 1.0
            rotm[blk + d + 16, blk + d] = 1.0
    tri = np.triu(np.ones((P, P), np.float32))  # [j, r]: keep j <= r
    identf = np.eye(P, dtype=np.float32)
    blk = np.zeros((P, 2), np.float32)
    blk[0:64, 0] = 1.0
    blk[64:128, 1] = 1.0
    consts = {
        "tri": tri.astype(_BF16),
        "identb": identf.astype(_BF16),
        "identf": identf,
        "blk": blk.astype(_BF16),
        "blkTb": np.ascontiguousarray(blk.T).astype(_BF16),
        "ones1f": np.ones((1, P), np.float32),
        "onescf": np.ones((P, 1), np.float32),
        "onescb": np.ones((P, 1), _BF16),
        "rotm": rotm.astype(_BF16),
        "wqkv": np.ascontiguousarray(w_qkv, dtype=np.float32),
        "wout": np.ascontiguousarray(w_out, dtype=np.float32),
        "sinkv": np.ascontiguousarray(sink.reshape(1, C), dtype=np.float32),
    }
    in_maps = []
    perms = []
    for c in range(NCORES):
        b, half = c // 2, c % 2
        idx = _perm_for(half)
        perms.append((b, idx))
        m = dict(consts)
        m["xb"] = np.ascontiguousarray(x[b][idx], dtype=np.float32)
        m["qct"], m["qst"] = _full_tables(ct32, st32, idx[:RQ])
        m["kct"], m["kst"] = _full_tables(ct32, st32, idx)
        m["bnd"] = np.full((P, P), float(half), np.float32).astype(_BF16)
        in_maps.append(m)
    return in_maps, perms


def _run(inputs, trace=False):
    from concourse.bass_utils import run_bass_kernel_spmd

    nc = _get_program()
    in_maps, perms = _make_in_maps(**inputs)
    res = run_bass_kernel_spmd(
        nc, in_maps, core_ids=list(range(NCORES)), trace=trace
    )
    full = np.empty((B, S, C), np.float32)
    for c in range(NCORES):
        b, idx = perms[c]
        full[b][idx[:RQ]] = res.results[c]["out"]
    return full, res


def kernel(x, re, attn_mask, w_qkv, w_out, sink):
    inputs = dict(x=np.asarray(x), re=np.asarray(re),
                  attn_mask=np.asarray(attn_mask),
                  w_qkv=np.asarray(w_qkv), w_out=np.asarray(w_out),
                  sink=np.asarray(sink))
    full, _ = _run(inputs)
    return full
